# revision 1
# baseline (speedup 1.0000x reference)
"""CTC loss Trainium2 kernel (Bass/Tile), pure data-parallel over 8 NeuronCores.

Contract: kernel(y_true [2048,32] i32, y_pred [2048,256,128] f32) -> loss [2048] f32.

Algorithm per core (256 examples = 2 blocks of 128, examples on partitions):
  Stream y in 32 tiles [128=(8 ex, 16 t_hi), 2048=(16 t_lo, 128 c)]:
    E = exp(y)                     ACT -> bf16
    Z[b,t] = sum_c E               DVE halve-add (bf16 2x) + reduce
    gather y at the 33 classes     GPSIMD ap_gather (f32, per-16-partition idx)
    fold gather -> q[128 ex, t*33+j]  one SBUF->SBUF DMA per tile
  One Ln pass over all Z + per-tile sums; partition-fold via small DRAM bounce.
  Per block of 128 examples:
    p'' = exp(q + PBIAS)           ACT -> bf16  (PBIAS recenters alpha drift)
    CTC forward DP as 65 tensor_tensor_scan ops along time t=0..255:
      alpha_t[s] = (u_t + alpha_{t-1}[s]) * p''_t[s]
      u = alpha[s-1] (+ mask_i * alpha[s-2] for odd s>=3) via scalar_tensor_tensor;
      t=0 init injected through a one-hot drive column.
    loss = sum_t log Z_t + T*PBIAS - log(alpha_T[63] + alpha_T[64])
"""
import contextlib
import sys

sys.path.insert(0, "/opt/trn_rl_repo")
import numpy as np

import concourse.bacc as bacc
import concourse.mybir as mybir
import concourse.tile as tile
from concourse import bass_utils

F32 = mybir.dt.float32
BF16 = mybir.dt.bfloat16
I16 = mybir.dt.int16
AOP = mybir.AluOpType
AF = mybir.ActivationFunctionType

N_CORES = 8
B_FULL = 2048
C = 128
T = 256
L = 32
NJ = L + 1       # 33 gathered classes: [blank] + labels
S = 2 * L + 1    # 65 CTC states
NEX = 8          # examples per tile
THI = 16         # t-high bits on partitions
TLO = 16         # t-low bits in free dim
TPB = 16         # tiles per block
N_BLOCKS = 2     # blocks per core (256 examples)
B_CORE = N_BLOCKS * 128
PBIAS = -0.95    # p'' = exp(y + PBIAS); balances FTZ-dip vs overflow margins
LOSS_CONST = -T * PBIAS


NIDX_G = 48  # 33 classes + 15 pad; ap_gather needs num_idxs % 16 == 0


def make_gidx(y_true_shard: np.ndarray) -> np.ndarray:
    """[ntiles, 128, NIDX_G//16] int16 ap_gather class indices (d=16 rows),
    p-major wrapped per 16-partition core group."""
    B = y_true_shard.shape[0]
    ntiles = B // NEX
    cls = np.concatenate(
        [np.zeros((B, 1), np.int64), y_true_shard.astype(np.int64)], axis=1
    )
    gidx = np.zeros((ntiles, 128, NIDX_G // 16), np.int16)
    i = np.arange(NIDX_G)
    prow, pcol = i % 16, i // 16
    for k in range(ntiles):
        for e in range(NEX):
            vals = np.zeros(NIDX_G, np.int64)
            vals[:NJ] = cls[k * NEX + e]
            gidx[k, 16 * e + prow, pcol] = vals
    return gidx


def make_mask(y_true_shard: np.ndarray) -> np.ndarray:
    """[nblocks, 128, L] f32 skip masks: mask[b, i] = (lab_i != lab_{i-1})."""
    B = y_true_shard.shape[0]
    m = np.zeros((B, L), np.float32)
    m[:, 1:] = (y_true_shard[:, 1:] != y_true_shard[:, :-1]).astype(np.float32)
    return m.reshape(B // 128, 128, L)


def scan3d(nc, out, data0, data1, initial, op0, op1):
    """tensor_tensor_scan allowing a 3-D strided data1 AP (verified on HW:
    the recurrence chains across AP dims in enumeration order)."""
    eng = nc.vector
    return eng.add_instruction(
        mybir.InstTensorScalarPtr(
            name=nc.get_next_instruction_name(),
            is_tensor_tensor_scan=True,
            is_scalar_tensor_tensor=True,
            op0=op0,
            op1=op1,
            ins=[
                eng.lower_ap(data0),
                eng.lower_ap_or_imm(initial),
                eng.lower_ap(data1),
            ],
            outs=[eng.lower_ap(out)],
        )
    )


def build_ctc(nc, loss_out, y_in, gidx_in, mask_in, n_blocks=N_BLOCKS, repeat=1,
              parts=("z", "gather", "fold", "dp")):
    ntiles = n_blocks * TPB
    with tile.TileContext(nc) as tc:
        with (
            tc.tile_pool(name="io", bufs=3) as io_pool,
            tc.tile_pool(name="ztmp", bufs=2) as z_pool,
            tc.tile_pool(name="gat", bufs=2) as g_pool,
            tc.tile_pool(name="qraw", bufs=2) as q_pool,
            tc.tile_pool(name="persist", bufs=1) as pp,
            tc.tile_pool(name="ubuf", bufs=2) as u_pool,
            tc.tile_pool(name="dram", bufs=1, space="DRAM") as d_pool,
        ):
            logzacc = pp.tile([128, ntiles], F32, tag="logzacc")
            zall = pp.tile([128, ntiles * TLO], F32, tag="zall")
            lnz = pp.tile([128, ntiles * TLO], F32, tag="lnz")
            scratch = d_pool.tile([ntiles * 128], F32, tag="scratch")
            dpA = pp.tile([128, T + 1], BF16, tag="dpA")
            dpB = pp.tile([128, T + 1], BF16, tag="dpB")
            cde = [
                pp.tile([128, T + 1], BF16, tag=f"cde{i}", name=f"cde{i}")
                for i in range(3)
            ]
            onehot = pp.tile([128, T], BF16, tag="onehot")
            masks = [
                pp.tile([128, L], F32, tag=f"mask{nb}", name=f"mask{nb}")
                for nb in range(n_blocks)
            ]
            lzsum = [
                pp.tile([128, 1], F32, tag=f"lzsum{nb}", name=f"lzsum{nb}")
                for nb in range(n_blocks)
            ]
            biasln = pp.tile([128, 1], F32, tag="biasln")

            nc.vector.memset(biasln[:], PBIAS)
            nc.vector.memset(onehot[:], 0.0)
            nc.vector.memset(onehot[:, 0:1], 1.0)
            for b in (dpA, dpB, *cde):
                nc.vector.memset(b[:, 0:1], 0.0)
            for nb in range(n_blocks):
                nc.sync.dma_start(masks[nb][:], mask_in[nb])

            y4 = y_in.rearrange("(k e) t c -> k e t c", e=NEX)

            loop_cm = (
                tc.For_i(0, repeat, 1) if repeat > 1 else contextlib.nullcontext()
            )
            with loop_cm:
                # ---- streaming phase ----
                qraws = []
                for nb in range(n_blocks):
                    q_raw = q_pool.tile([128, T * NJ], BF16, tag="qraw", name="qraw")
                    qraws.append(q_raw)
                    for kl in range(TPB):
                        k = nb * TPB + kl
                        ty = io_pool.tile([128, TLO * C], F32, tag="y")
                        src = y4[k].rearrange("e (th tl) c -> (e th) (tl c)", th=THI)
                        nc.sync.dma_start(ty[:], src)

                        te = z_pool.tile([128, TLO * C], BF16, tag="E")
                        # write E = exp(y + PBIAS) transposed to (c, tl)-major
                        te_t = te[:].rearrange("p (c tl) -> p tl c", tl=TLO)
                        ty3 = ty[:].rearrange("p (tl c) -> p tl c", tl=TLO)
                        nc.scalar.activation(
                            te_t, ty3, AF.Exp, bias=biasln[:, 0:1]
                        )
                        if "z" not in parts:
                            continue
                        e3c = te[:].rearrange("p (c tl) -> p c tl", c=C)
                        th = z_pool.tile([128, 64 * TLO], BF16, tag="H")
                        h3 = th[:].rearrange("p (c tl) -> p c tl", c=64)
                        nc.vector.tensor_tensor(
                            out=h3, in0=e3c[:, 0:64, :], in1=e3c[:, 64:128, :],
                            op=AOP.add,
                        )
                        h3r = th[:].rearrange("p (c tl) -> p tl c", c=64)
                        nc.vector.tensor_reduce(
                            out=zall[:, k * TLO : (k + 1) * TLO], in_=h3r,
                            axis=mybir.AxisListType.X, op=AOP.add,
                        )

                        if "gather" not in parts:
                            continue
                        tidx = g_pool.tile([128, NIDX_G // 16], I16, tag="idx")
                        nc.sync.dma_start(tidx[:], gidx_in[k])
                        tgat = g_pool.tile([128, NIDX_G * TLO], BF16, tag="gat")
                        nc.gpsimd.ap_gather(
                            out_ap=tgat[:],
                            in_ap=te[:].rearrange("p (c tl) -> p c tl", c=C),
                            idxs_ap=tidx[:],
                            channels=128,
                            num_elems=C,
                            d=TLO,
                            num_idxs=NIDX_G,
                        )
                        if "fold" not in parts:
                            continue
                        # fold: (e,th) parts x (j,tl) -> [8 ex, th*528 + j*16 + tl]
                        dst = q_raw[:].rearrange("p (th w) -> p th w", th=THI)
                        nc.scalar.dma_start(
                            dst[kl * NEX : (kl + 1) * NEX], tgat[:, 0 : NJ * TLO]
                        )

                    if "fold" not in parts:
                        nc.vector.memset(qraws[nb][:, 0:1], 0.0)

                # ---- logZ: one Ln pass, per-tile sums, fold via DRAM bounce ----
                if "z" not in parts:
                    nc.vector.memset(zall[:], 1.0)
                nc.scalar.activation(lnz[:], zall[:], AF.Ln)
                lnz3 = lnz[:].rearrange("p (k tl) -> p k tl", tl=TLO)
                nc.vector.tensor_reduce(
                    out=logzacc[:], in_=lnz3, axis=mybir.AxisListType.X, op=AOP.add
                )
                sc3 = scratch[:].rearrange("(k e th) -> e th k", e=NEX, th=THI)
                nc.sync.dma_start(sc3, logzacc[:])
                for nb in range(n_blocks):
                    tlzs = z_pool.tile([128, THI], F32, tag="lzread")
                    rb = scratch[:].rearrange(
                        "(nb kl e th) -> nb (kl e) th", nb=n_blocks, kl=TPB, e=NEX
                    )
                    nc.sync.dma_start(tlzs[:], rb[nb])
                    nc.vector.tensor_reduce(
                        out=lzsum[nb][:], in_=tlzs[:], axis=mybir.AxisListType.X,
                        op=AOP.add,
                    )

                # ---- DP phase ----
                for nb in range(n_blocks if "dp" in parts else 0):
                    p3 = qraws[nb][:].rearrange(
                        "p (th j tl) -> p th j tl", th=THI, j=NJ
                    )

                    def pcol(s):
                        j = 0 if s % 2 == 0 else 1 + (s - 1) // 2
                        return p3[:, :, j, :]

                    scan3d(
                        nc, dpA[:, 1 : T + 1], onehot[:], pcol(0), 0.0,
                        AOP.add, AOP.mult,
                    )
                    tu1 = u_pool.tile([128, T], BF16, tag="U", name="tu1")
                    nc.vector.scalar_tensor_tensor(
                        out=tu1[:], in0=onehot[:], scalar=1.0, in1=dpA[:, 0:T],
                        op0=AOP.mult, op1=AOP.add,
                    )
                    scan3d(
                        nc, dpB[:, 1 : T + 1], tu1[:], pcol(1), 0.0,
                        AOP.add, AOP.mult,
                    )
                    prev2, prev1 = dpA, dpB
                    for s in range(2, S):
                        cur = cde[(s - 2) % 3]
                        if s % 2 == 0:
                            d0 = prev1[:, 0:T]
                        else:
                            i = (s - 1) // 2
                            tu = u_pool.tile([128, T], BF16, tag="U")
                            nc.vector.scalar_tensor_tensor(
                                out=tu[:], in0=prev2[:, 0:T],
                                scalar=masks[nb][:, i : i + 1],
                                in1=prev1[:, 0:T], op0=AOP.mult, op1=AOP.add,
                            )
                            d0 = tu[:]
                        scan3d(
                            nc, cur[:, 1 : T + 1], d0, pcol(s), 0.0,
                            AOP.add, AOP.mult,
                        )
                        prev2, prev1 = prev1, cur

                    fin = z_pool.tile([128, 1], F32, tag="fin")
                    nc.vector.tensor_tensor(
                        out=fin[:], in0=prev1[:, T : T + 1],
                        in1=prev2[:, T : T + 1], op=AOP.add,
                    )
                    lfin = z_pool.tile([128, 1], F32, tag="lfin")
                    nc.scalar.activation(lfin[:], fin[:], AF.Ln)
                    tloss = z_pool.tile([128, 1], F32, tag="loss")
                    nc.vector.tensor_tensor(
                        out=tloss[:], in0=lzsum[nb][:], in1=lfin[:], op=AOP.subtract
                    )
                    nc.sync.dma_start(
                        loss_out[nb * 128 : (nb + 1) * 128], tloss[:, 0:1]
                    )


def _build_program(repeat=1, parts=("z", "gather", "fold", "dp")):
    nc = bacc.Bacc("TRN2", num_devices=N_CORES, enable_partition_id=False)
    y_in = nc.dram_tensor("y", [B_CORE, T, C], F32, kind="ExternalInput").ap()
    gidx_in = nc.dram_tensor(
        "gidx", [N_BLOCKS * TPB, 128, NIDX_G // 16], I16, kind="ExternalInput"
    ).ap()
    mask_in = nc.dram_tensor(
        "mask", [N_BLOCKS, 128, L], F32, kind="ExternalInput"
    ).ap()
    loss_out = nc.dram_tensor("loss", [B_CORE], F32, kind="ExternalOutput").ap()
    build_ctc(nc, loss_out, y_in, gidx_in, mask_in, repeat=repeat, parts=parts)
    nc.compile()
    return nc


def kernel(y_true: np.ndarray, y_pred: np.ndarray):
    y_true = np.asarray(y_true)
    y_pred = np.ascontiguousarray(np.asarray(y_pred, dtype=np.float32))
    assert y_pred.shape == (B_FULL, T, C) and y_true.shape == (B_FULL, L)

    nc = _build_program()
    in_maps = []
    for core in range(N_CORES):
        sl = slice(core * B_CORE, (core + 1) * B_CORE)
        yt = y_true[sl]
        in_maps.append(
            {
                "y": y_pred[sl],
                "gidx": make_gidx(yt),
                "mask": make_mask(yt),
            }
        )
    res = bass_utils.run_bass_kernel_spmd(
        nc, in_maps, core_ids=list(range(N_CORES))
    )
    loss = np.concatenate([r["loss"] for r in res.results])
    return loss.astype(np.float32)


if __name__ == "__main__":
    rng = np.random.default_rng(0)
    yp = rng.standard_normal((B_FULL, T, C)).astype(np.float32)
    yt = rng.integers(1, C, (B_FULL, L)).astype(np.int32)
    out = kernel(yt, yp)
    print(out.shape, out[:4])



# revision 21
# speedup vs baseline: 209.2248x; 209.2248x over previous
"""CTC loss Trainium2 kernel (Bass/Tile), pure data-parallel over 8 NeuronCores.

Contract: kernel(y_true [2048,32] i32, y_pred [2048,256,128] f32) -> loss [2048] f32.

Algorithm per core (256 examples = 2 blocks of 128, examples on partitions):
  Stream y in 32 tiles [128=(8 ex, 16 t_hi), 2048=(16 t_lo, 128 c)]:
    E = exp(y + PBIAS)             ACT -> bf16, written (c, tl)-major
    Z[b,t] = sum_c E               DVE: two halve-adds (bf16 2x) + reduce-32
    gather E at the 33 classes     GPSIMD ap_gather (per-16-partition idx)
    fold gather -> q[128 ex, th*528 + j*16 + tl]   SBUF->SBUF DMA (PE queue)
  DP(block0) emitted right after block0's 16 tiles so it overlaps block1
  streaming; only the tiny final-column adds run there, Ln/subtract deferred.
  logZ: one Ln pass over all Z, per-tile sums, then a PE matmul against a
  one-hot [128,8] selector folds the 16 t_hi partitions per example
  (replaces the strided DRAM bounce); tiny [8,32] bounce for the partition
  rearrange back to [128,1] per block.
  Per block of 128 examples:
    p'' = exp(q + PBIAS)  (PBIAS recenters alpha drift)
    CTC forward DP as 65 tensor_tensor_scan ops along time t=0..255:
      alpha_t[s] = (u_t + alpha_{t-1}[s]) * p''_t[s]
      u = alpha[s-1] (+ mask_i * alpha[s-2] for odd s>=3) via scalar_tensor_tensor;
      t=0 init injected through a one-hot drive column.
    loss = sum_t log Z_t + T*PBIAS - log(alpha_T[63] + alpha_T[64])
"""
import contextlib
import sys

sys.path.insert(0, "/opt/trn_rl_repo")
import numpy as np

import concourse.bacc as bacc
import concourse.mybir as mybir
import concourse.tile as tile
from concourse import bass_utils

F32 = mybir.dt.float32
BF16 = mybir.dt.bfloat16
I16 = mybir.dt.int16
AOP = mybir.AluOpType
AF = mybir.ActivationFunctionType

N_CORES = 8
B_FULL = 2048
C = 128
T = 256
L = 32
NJ = L + 1       # 33 gathered classes: [blank] + labels
S = 2 * L + 1    # 65 CTC states
NEX = 8          # examples per tile
THI = 16         # t-high bits on partitions
TLO = 16         # t-low bits in free dim
TPB = 16         # tiles per block
N_BLOCKS = 2     # blocks per core (256 examples)
B_CORE = N_BLOCKS * 128
NTILES = N_BLOCKS * TPB
PBIAS = -0.95    # p'' = exp(y + PBIAS); balances FTZ-dip vs overflow margins
LOSS_CONST = -T * PBIAS


NIDX_G = 48  # 33 classes + 15 pad; ap_gather idx table is 16-partition-wrapped


GIDX_STRIDE = 8  # i16 cols per tile: 3 used + 5 pad so each tile's idx
                 # slice starts 16-byte aligned (HW gather requirement)


def make_gidx(y_true_shard: np.ndarray) -> np.ndarray:
    """[128, ntiles * GIDX_STRIDE] int16 ap_gather class indices, all tiles
    packed per partition (16B-aligned slices) so one DMA loads everything."""
    B = y_true_shard.shape[0]
    ntiles = B // NEX
    cls = np.concatenate(
        [np.zeros((B, 1), np.int64), y_true_shard.astype(np.int64)], axis=1
    )
    gidx = np.zeros((128, ntiles * GIDX_STRIDE), np.int16)
    i = np.arange(NIDX_G)
    prow, pcol = i % 16, i // 16
    for k in range(ntiles):
        for e in range(NEX):
            vals = np.zeros(NIDX_G, np.int64)
            vals[:NJ] = cls[k * NEX + e]
            gidx[16 * e + prow, k * GIDX_STRIDE + pcol] = vals
    return gidx


def make_sel() -> np.ndarray:
    """[128, NEX] f32 one-hot: sel[p, e] = (p // 16 == e); PE-matmul selector
    that folds the 16 t_hi partitions of each example."""
    p = np.arange(128)
    return (p[:, None] // 16 == np.arange(NEX)[None, :]).astype(np.float32)


def make_mask(y_true_shard: np.ndarray) -> np.ndarray:
    """[nblocks, 128, L] f32 skip masks: mask[b, i] = (lab_i != lab_{i-1})."""
    B = y_true_shard.shape[0]
    m = np.zeros((B, L), np.float32)
    m[:, 1:] = (y_true_shard[:, 1:] != y_true_shard[:, :-1]).astype(np.float32)
    return m.reshape(B // 128, 128, L)


def scan3d(nc, out, data0, data1, initial, op0, op1):
    """tensor_tensor_scan allowing a 3-D strided data1 AP (verified on HW:
    the recurrence chains across AP dims in enumeration order)."""
    eng = nc.vector
    return eng.add_instruction(
        mybir.InstTensorScalarPtr(
            name=nc.get_next_instruction_name(),
            is_tensor_tensor_scan=True,
            is_scalar_tensor_tensor=True,
            op0=op0,
            op1=op1,
            ins=[
                eng.lower_ap(data0),
                eng.lower_ap_or_imm(initial),
                eng.lower_ap(data1),
            ],
            outs=[eng.lower_ap(out)],
        )
    )


def build_ctc(nc, loss_out, y_in, gidx_in, mask_in, sel_in, n_blocks=N_BLOCKS, repeat=1):
    ntiles = n_blocks * TPB
    with tile.TileContext(nc) as tc:
        with (
            tc.tile_pool(name="io", bufs=3) as io_pool,
            tc.tile_pool(name="ztmp", bufs=2) as z_pool,
            tc.tile_pool(name="gat", bufs=2) as g_pool,
            tc.tile_pool(name="qraw", bufs=2) as q_pool,
            tc.tile_pool(name="persist", bufs=1) as pp,
            tc.tile_pool(name="ubuf", bufs=2) as u_pool,
            tc.tile_pool(name="dram", bufs=1, space="DRAM") as d_pool,
            tc.psum_pool(name="psum", bufs=1) as ps_pool,
        ):
            logzacc = pp.tile([128, ntiles], F32, tag="logzacc")
            zall = pp.tile([128, ntiles * TLO], F32, tag="zall")
            lnz = pp.tile([128, ntiles * TLO], F32, tag="lnz")
            scratch = d_pool.tile([NEX * ntiles], F32, tag="scratch")
            gidx_all = pp.tile([128, ntiles * GIDX_STRIDE], I16, tag="gidx")
            dpA = pp.tile([128, T + 1], BF16, tag="dpA")
            dpB = pp.tile([128, T + 1], BF16, tag="dpB")
            cde = [
                pp.tile([128, T + 1], BF16, tag=f"cde{i}", name=f"cde{i}")
                for i in range(3)
            ]
            onehot = pp.tile([128, T], BF16, tag="onehot")
            sel = pp.tile([128, NEX], F32, tag="sel")
            masks = [
                pp.tile([128, L], F32, tag=f"mask{nb}", name=f"mask{nb}")
                for nb in range(n_blocks)
            ]
            fins = pp.tile([128, n_blocks], F32, tag="fins")
            lzs = pp.tile([128, n_blocks], F32, tag="lzs")
            biasln = pp.tile([128, 1], F32, tag="biasln")
            zsum_ps = ps_pool.tile([NEX, ntiles], F32, tag="zsum")

            nc.vector.memset(biasln[:], PBIAS)
            nc.vector.memset(onehot[:], 0.0)
            nc.vector.memset(onehot[:, 0:1], 1.0)
            for b in (dpA, dpB, *cde):
                nc.vector.memset(b[:, 0:1], 0.0)
            for nb in range(n_blocks):
                nc.sync.dma_start(masks[nb][:], mask_in[nb])
            nc.sync.dma_start(gidx_all[:], gidx_in)
            nc.sync.dma_start(sel[:], sel_in)

            # Touch Exp and Ln once before the loop so activation-table
            # loads don't land inside the repeat body.
            warm = pp.tile([128, 1], F32, tag="warm")
            nc.scalar.activation(warm[:], biasln[:], AF.Exp)
            nc.scalar.activation(warm[:], warm[:], AF.Ln)

            y4 = y_in.rearrange("(k e) t c -> k e t c", e=NEX)

            loop_cm = (
                tc.For_i(0, repeat, 1) if repeat > 1 else contextlib.nullcontext()
            )
            with loop_cm:
                qraws = [
                    q_pool.tile([128, T * NJ], BF16, tag="qraw", name=f"qraw{nb}")
                    for nb in range(n_blocks)
                ]
                dps = []  # per block: (prev1, prev2) at the end of its chain

                def emit_tile(k, kl, nb):
                    ty = io_pool.tile([128, TLO * C], F32, tag="y", bufs=4)
                    src = y4[k].rearrange("e (th tl) c -> (e th) (tl c)", th=THI)
                    nc.sync.dma_start(ty[:], src)

                    # 8 E bufs: lets ACT/Pool run ahead while block1's Z ops
                    # queue behind DP(block0) on the DVE.
                    te = z_pool.tile([128, TLO * C], BF16, tag="E", bufs=int(os.environ.get("CTC_EBUFS", "8")))
                    # write E = exp(y + PBIAS) transposed to (c, tl)-major
                    te_t = te[:].rearrange("p (c tl) -> p tl c", tl=TLO)
                    ty3 = ty[:].rearrange("p (tl c) -> p tl c", tl=TLO)
                    nc.scalar.activation(te_t, ty3, AF.Exp, bias=biasln[:, 0:1])

                    # Z: halve-add 128->64->32, then reduce over 32
                    e3c = te[:].rearrange("p (c tl) -> p c tl", c=C)
                    th1 = z_pool.tile([128, 64 * TLO], BF16, tag="H")
                    h3 = th1[:].rearrange("p (c tl) -> p c tl", c=64)
                    nc.vector.tensor_tensor(
                        out=h3, in0=e3c[:, 0:64, :], in1=e3c[:, 64:128, :],
                        op=AOP.add,
                    )
                    th2 = z_pool.tile([128, 32 * TLO], BF16, tag="H2")
                    h23 = th2[:].rearrange("p (c tl) -> p c tl", c=32)
                    nc.vector.tensor_tensor(
                        out=h23, in0=h3[:, 0:32, :], in1=h3[:, 32:64, :],
                        op=AOP.add,
                    )
                    h2r = th2[:].rearrange("p (c tl) -> p tl c", c=32)
                    nc.vector.tensor_reduce(
                        out=zall[:, k * TLO : (k + 1) * TLO], in_=h2r,
                        axis=mybir.AxisListType.X, op=AOP.add,
                    )

                    tgat = g_pool.tile([128, NIDX_G * TLO], BF16, tag="gat")
                    ncol = NIDX_G // 16
                    nc.gpsimd.ap_gather(
                        out_ap=tgat[:],
                        in_ap=te[:].rearrange("p (c tl) -> p c tl", c=C),
                        idxs_ap=gidx_all[:, k * GIDX_STRIDE : k * GIDX_STRIDE + ncol],
                        channels=128,
                        num_elems=C,
                        d=TLO,
                        num_idxs=NIDX_G,
                    )
                    return tgat

                def emit_fold(kl, nb, tgat):
                    # fold: (e,th) parts x (j,tl) -> [8 ex, th*528 + j*16 + tl].
                    # Emitted a few tiles late so its wait on the gather is
                    # already satisfied when it reaches the SP queue head
                    # (otherwise it stalls the next y-load behind it).
                    dst = qraws[nb][:].rearrange("p (th w) -> p th w", th=THI)
                    nc.sync.dma_start(
                        dst[kl * NEX : (kl + 1) * NEX], tgat[:, 0 : NJ * TLO]
                    )

                def gen_dp_scans(nb):
                    """All-DVE part of the DP for block nb, one op per yield
                    so it can be interleaved with streaming Z ops; the final
                    Ln runs after all streaming ACT work."""
                    p3 = qraws[nb][:].rearrange(
                        "p (th j tl) -> p th j tl", th=THI, j=NJ
                    )

                    def pcol(s):
                        j = 0 if s % 2 == 0 else 1 + (s - 1) // 2
                        return p3[:, :, j, :]

                    scan3d(
                        nc, dpA[:, 1 : T + 1], onehot[:], pcol(0), 0.0,
                        AOP.add, AOP.mult,
                    )
                    yield
                    tu1 = u_pool.tile([128, T], BF16, tag="U", name=f"tu1_{nb}")
                    nc.vector.scalar_tensor_tensor(
                        out=tu1[:], in0=onehot[:], scalar=1.0, in1=dpA[:, 0:T],
                        op0=AOP.mult, op1=AOP.add,
                    )
                    yield
                    scan3d(
                        nc, dpB[:, 1 : T + 1], tu1[:], pcol(1), 0.0,
                        AOP.add, AOP.mult,
                    )
                    yield
                    prev2, prev1 = dpA, dpB
                    for s in range(2, S):
                        cur = cde[(s - 2) % 3]
                        if s % 2 == 0:
                            d0 = prev1[:, 0:T]
                        else:
                            i = (s - 1) // 2
                            tu = u_pool.tile([128, T], BF16, tag="U")
                            nc.vector.scalar_tensor_tensor(
                                out=tu[:], in0=prev2[:, 0:T],
                                scalar=masks[nb][:, i : i + 1],
                                in1=prev1[:, 0:T], op0=AOP.mult, op1=AOP.add,
                            )
                            yield
                            d0 = tu[:]
                        scan3d(
                            nc, cur[:, 1 : T + 1], d0, pcol(s), 0.0,
                            AOP.add, AOP.mult,
                        )
                        yield
                        prev2, prev1 = prev1, cur

                    # fin = alpha[S-1] + alpha[S-2] at t=T-1 (DVE, tiny)
                    nc.vector.tensor_tensor(
                        out=fins[:, nb : nb + 1], in0=prev1[:, T : T + 1],
                        in1=prev2[:, T : T + 1], op=AOP.add,
                    )
                    yield

                # ---- streaming (folds delayed 3 tiles); DP0 interleaved ----
                FOLD_LAG = 3
                N_DP_OPS = 2 * S + L  # 97 DVE ops per block's DP
                pending = []
                dp0 = None
                for k in range(ntiles):
                    nb, kl = divmod(k, TPB)
                    pending.append((kl, nb, emit_tile(k, kl, nb)))
                    if len(pending) > FOLD_LAG:
                        emit_fold(*pending.pop(0))
                        if k - FOLD_LAG == TPB - 1:
                            dp0 = gen_dp_scans(0)
                    if dp0 is not None:
                        # ~8 DP ops per remaining block1 tile keeps the DVE
                        # queue alternating between DP(b0) and Z(b1) work.
                        per_tile = -(-N_DP_OPS // (ntiles - 1 - (TPB + FOLD_LAG - 1)))
                        for _ in range(per_tile):
                            if next(dp0, StopIteration) is StopIteration:
                                dp0 = None
                                break
                while dp0 is not None and next(dp0, StopIteration) is not StopIteration:
                    pass
                for args in pending:
                    emit_fold(*args)

                # ---- logZ: Ln pass, per-tile sums, PE-matmul partition fold
                nc.scalar.activation(lnz[:], zall[:], AF.Ln)
                lnz3 = lnz[:].rearrange("p (k tl) -> p k tl", tl=TLO)
                nc.vector.tensor_reduce(
                    out=logzacc[:], in_=lnz3, axis=mybir.AxisListType.X, op=AOP.add
                )
                # zsum_ps[e, k] = sum_th logzacc[(e,th), k] = lzsum(ex = k*8+e)
                nc.tensor.matmul(
                    out=zsum_ps[:], lhsT=sel[:], rhs=logzacc[:],
                    start=True, stop=True,
                )
                zsum_sb = z_pool.tile([NEX, ntiles], F32, tag="zsum_sb")
                nc.scalar.copy(zsum_sb[:], zsum_ps[:])
                sc_w = scratch[:].rearrange("(e k) -> e k", e=NEX)
                nc.sync.dma_start(sc_w, zsum_sb[:])
                # rb[nb] enumerates (kl, e) = block-local example order
                rb = scratch[:].rearrange(
                    "(e nb kl) -> nb kl e", e=NEX, nb=n_blocks
                )
                for nb in range(n_blocks):
                    nc.sync.dma_start(lzs[:, nb : nb + 1], rb[nb])

                for _ in gen_dp_scans(1):
                    pass

                # ---- final loss: one Ln over both blocks' fins, subtract ----
                lfin = z_pool.tile([128, n_blocks], F32, tag="lfin")
                nc.scalar.activation(lfin[:], fins[:], AF.Ln)
                tloss = z_pool.tile([128, n_blocks], F32, tag="loss")
                nc.vector.tensor_tensor(
                    out=tloss[:], in0=lzs[:], in1=lfin[:], op=AOP.subtract
                )
                for nb in range(n_blocks):
                    nc.sync.dma_start(
                        loss_out[nb * 128 : (nb + 1) * 128], tloss[:, nb : nb + 1]
                    )


def _force_combined_act_table(nc):
    """Trim Exp/Ln from all act-function sets except the one that holds both,
    so the table-load pass picks the combined set and never reloads between
    the Exp (streaming) and Ln (logZ / final) activations."""
    from concourse.hw_specs import get_activation_tables

    tabs = get_activation_tables(nc.m.arch)
    combined = None
    for name, s in tabs.items():
        if AF.Exp in s and AF.Ln in s:
            combined = name
            break
    if combined is None:
        return
    for name, s in tabs.items():
        if name != combined:
            s.discard(AF.Exp)
            s.discard(AF.Ln)


import os

def _build_program(repeat=1):
    nc = bacc.Bacc("TRN2", num_devices=N_CORES, enable_partition_id=False)
    if not os.environ.get("CTC_NO_ACTFIX"):
        _force_combined_act_table(nc)
    y_in = nc.dram_tensor("y", [B_CORE, T, C], F32, kind="ExternalInput").ap()
    gidx_in = nc.dram_tensor(
        "gidx", [128, NTILES * GIDX_STRIDE], I16, kind="ExternalInput"
    ).ap()
    mask_in = nc.dram_tensor(
        "mask", [N_BLOCKS, 128, L], F32, kind="ExternalInput"
    ).ap()
    sel_in = nc.dram_tensor("sel", [128, NEX], F32, kind="ExternalInput").ap()
    loss_out = nc.dram_tensor("loss", [B_CORE], F32, kind="ExternalOutput").ap()
    build_ctc(nc, loss_out, y_in, gidx_in, mask_in, sel_in, repeat=repeat)
    nc.compile()
    return nc


def kernel(y_true: np.ndarray, y_pred: np.ndarray):
    y_true = np.asarray(y_true)
    y_pred = np.ascontiguousarray(np.asarray(y_pred, dtype=np.float32))
    assert y_pred.shape == (B_FULL, T, C) and y_true.shape == (B_FULL, L)

    nc = _build_program()
    in_maps = []
    for core in range(N_CORES):
        sl = slice(core * B_CORE, (core + 1) * B_CORE)
        yt = y_true[sl]
        in_maps.append(
            {
                "y": y_pred[sl],
                "gidx": make_gidx(yt),
                "mask": make_mask(yt),
                "sel": make_sel(),
            }
        )
    res = bass_utils.run_bass_kernel_spmd(
        nc, in_maps, core_ids=list(range(N_CORES))
    )
    loss = np.concatenate([r["loss"] for r in res.results])
    return loss.astype(np.float32)


if __name__ == "__main__":
    rng = np.random.default_rng(0)
    yp = rng.standard_normal((B_FULL, T, C)).astype(np.float32)
    yt = rng.integers(1, C, (B_FULL, L)).astype(np.int32)
    out = kernel(yt, yp)
    print(out.shape, out[:4])


# revision 22
# speedup vs baseline: 210.4326x; 1.0058x over previous
"""CTC loss Trainium2 kernel (Bass/Tile), pure data-parallel over 8 NeuronCores.

Contract: kernel(y_true [2048,32] i32, y_pred [2048,256,128] f32) -> loss [2048] f32.

Algorithm per core (256 examples = 2 blocks of 128, examples on partitions):
  Stream y in 32 tiles [128=(8 ex, 16 t_hi), 2048=(16 t_lo, 128 c)]:
    E = exp(y + PBIAS)             ACT -> bf16, written (c, tl)-major
    Z[b,t] = sum_c E               DVE: two halve-adds (bf16 2x) + reduce-32
    gather E at the 33 classes     GPSIMD ap_gather (per-16-partition idx)
    fold gather -> q[128 ex, th*528 + j*16 + tl]   SBUF->SBUF DMA (PE queue)
  DP(block0) emitted right after block0's 16 tiles so it overlaps block1
  streaming; only the tiny final-column adds run there, Ln/subtract deferred.
  logZ: one Ln pass over all Z, per-tile sums, then a PE matmul against a
  one-hot [128,8] selector folds the 16 t_hi partitions per example
  (replaces the strided DRAM bounce); tiny [8,32] bounce for the partition
  rearrange back to [128,1] per block.
  Per block of 128 examples:
    p'' = exp(q + PBIAS)  (PBIAS recenters alpha drift)
    CTC forward DP as 65 tensor_tensor_scan ops along time t=0..255:
      alpha_t[s] = (u_t + alpha_{t-1}[s]) * p''_t[s]
      u = alpha[s-1] (+ mask_i * alpha[s-2] for odd s>=3) via scalar_tensor_tensor;
      t=0 init injected through a one-hot drive column.
    loss = sum_t log Z_t + T*PBIAS - log(alpha_T[63] + alpha_T[64])
"""
import contextlib
import sys

sys.path.insert(0, "/opt/trn_rl_repo")
import numpy as np

import concourse.bacc as bacc
import concourse.mybir as mybir
import concourse.tile as tile
from concourse import bass_utils

F32 = mybir.dt.float32
BF16 = mybir.dt.bfloat16
I16 = mybir.dt.int16
AOP = mybir.AluOpType
AF = mybir.ActivationFunctionType

N_CORES = 8
B_FULL = 2048
C = 128
T = 256
L = 32
NJ = L + 1       # 33 gathered classes: [blank] + labels
S = 2 * L + 1    # 65 CTC states
NEX = 8          # examples per tile
THI = 16         # t-high bits on partitions
TLO = 16         # t-low bits in free dim
TPB = 16         # tiles per block
N_BLOCKS = 2     # blocks per core (256 examples)
B_CORE = N_BLOCKS * 128
NTILES = N_BLOCKS * TPB
PBIAS = -0.95    # p'' = exp(y + PBIAS); balances FTZ-dip vs overflow margins
LOSS_CONST = -T * PBIAS


NIDX_G = 48  # 33 classes + 15 pad; ap_gather idx table is 16-partition-wrapped


GIDX_STRIDE = 8  # i16 cols per tile: 3 used + 5 pad so each tile's idx
                 # slice starts 16-byte aligned (HW gather requirement)


def make_gidx(y_true_shard: np.ndarray) -> np.ndarray:
    """[128, ntiles * GIDX_STRIDE] int16 ap_gather class indices, all tiles
    packed per partition (16B-aligned slices) so one DMA loads everything."""
    B = y_true_shard.shape[0]
    ntiles = B // NEX
    cls = np.concatenate(
        [np.zeros((B, 1), np.int64), y_true_shard.astype(np.int64)], axis=1
    )
    gidx = np.zeros((128, ntiles * GIDX_STRIDE), np.int16)
    i = np.arange(NIDX_G)
    prow, pcol = i % 16, i // 16
    for k in range(ntiles):
        for e in range(NEX):
            vals = np.zeros(NIDX_G, np.int64)
            vals[:NJ] = cls[k * NEX + e]
            gidx[16 * e + prow, k * GIDX_STRIDE + pcol] = vals
    return gidx


def make_sel() -> np.ndarray:
    """[128, NEX] f32 one-hot: sel[p, e] = (p // 16 == e); PE-matmul selector
    that folds the 16 t_hi partitions of each example."""
    p = np.arange(128)
    return (p[:, None] // 16 == np.arange(NEX)[None, :]).astype(np.float32)


def make_mask(y_true_shard: np.ndarray) -> np.ndarray:
    """[nblocks, 128, L] f32 skip masks: mask[b, i] = (lab_i != lab_{i-1})."""
    B = y_true_shard.shape[0]
    m = np.zeros((B, L), np.float32)
    m[:, 1:] = (y_true_shard[:, 1:] != y_true_shard[:, :-1]).astype(np.float32)
    return m.reshape(B // 128, 128, L)


def scan3d(nc, out, data0, data1, initial, op0, op1):
    """tensor_tensor_scan allowing a 3-D strided data1 AP (verified on HW:
    the recurrence chains across AP dims in enumeration order)."""
    eng = nc.vector
    return eng.add_instruction(
        mybir.InstTensorScalarPtr(
            name=nc.get_next_instruction_name(),
            is_tensor_tensor_scan=True,
            is_scalar_tensor_tensor=True,
            op0=op0,
            op1=op1,
            ins=[
                eng.lower_ap(data0),
                eng.lower_ap_or_imm(initial),
                eng.lower_ap(data1),
            ],
            outs=[eng.lower_ap(out)],
        )
    )


def build_ctc(nc, loss_out, y_in, gidx_in, mask_in, sel_in, n_blocks=N_BLOCKS, repeat=1):
    ntiles = n_blocks * TPB
    with tile.TileContext(nc) as tc:
        with (
            tc.tile_pool(name="io", bufs=3) as io_pool,
            tc.tile_pool(name="ztmp", bufs=2) as z_pool,
            tc.tile_pool(name="gat", bufs=2) as g_pool,
            tc.tile_pool(name="qraw", bufs=2) as q_pool,
            tc.tile_pool(name="persist", bufs=1) as pp,
            tc.tile_pool(name="ubuf", bufs=2) as u_pool,
            tc.tile_pool(name="dram", bufs=1, space="DRAM") as d_pool,
            tc.psum_pool(name="psum", bufs=1) as ps_pool,
        ):
            logzacc = pp.tile([128, ntiles], F32, tag="logzacc")
            zall = pp.tile([128, ntiles * TLO], F32, tag="zall")
            lnz = pp.tile([128, ntiles * TLO], F32, tag="lnz")
            scratch = d_pool.tile([NEX * ntiles], F32, tag="scratch")
            gidx_all = pp.tile([128, ntiles * GIDX_STRIDE], I16, tag="gidx")
            dpA = pp.tile([128, T + 1], BF16, tag="dpA")
            dpB = pp.tile([128, T + 1], BF16, tag="dpB")
            cde = [
                pp.tile([128, T + 1], BF16, tag=f"cde{i}", name=f"cde{i}")
                for i in range(3)
            ]
            onehot = pp.tile([128, T], BF16, tag="onehot")
            sel = pp.tile([128, NEX], F32, tag="sel")
            masks = [
                pp.tile([128, L], F32, tag=f"mask{nb}", name=f"mask{nb}")
                for nb in range(n_blocks)
            ]
            fins = pp.tile([128, n_blocks], F32, tag="fins")
            lzs = pp.tile([128, n_blocks], F32, tag="lzs")
            biasln = pp.tile([128, 1], F32, tag="biasln")
            zsum_ps = ps_pool.tile([NEX, ntiles], F32, tag="zsum")

            nc.vector.memset(biasln[:], PBIAS)
            nc.vector.memset(onehot[:], 0.0)
            nc.vector.memset(onehot[:, 0:1], 1.0)
            for b in (dpA, dpB, *cde):
                nc.vector.memset(b[:, 0:1], 0.0)
            for nb in range(n_blocks):
                nc.sync.dma_start(masks[nb][:], mask_in[nb])
            nc.sync.dma_start(gidx_all[:], gidx_in)
            nc.sync.dma_start(sel[:], sel_in)

            # Touch Exp and Ln once before the loop so activation-table
            # loads don't land inside the repeat body.
            warm = pp.tile([128, 1], F32, tag="warm")
            nc.scalar.activation(warm[:], biasln[:], AF.Exp)
            nc.scalar.activation(warm[:], warm[:], AF.Ln)
            # Touch ap_gather too: hoists the gpsimd library load (which
            # otherwise lands inside the loop body, behind the per-iteration
            # all-engine barrier).
            warmg_in = pp.tile([128, 2 * C], BF16, tag="warmg_in")
            warmg_out = pp.tile([128, 2 * NIDX_G], BF16, tag="warmg_out")
            nc.vector.memset(warmg_in[:], 0.0)
            nc.gpsimd.ap_gather(
                out_ap=warmg_out[:],
                in_ap=warmg_in[:].rearrange("p (c d) -> p c d", c=C),
                idxs_ap=gidx_all[:, 0 : NIDX_G // 16],
                channels=128,
                num_elems=C,
                d=2,
                num_idxs=NIDX_G,
            )

            y4 = y_in.rearrange("(k e) t c -> k e t c", e=NEX)

            loop_cm = (
                tc.For_i(0, repeat, 1) if repeat > 1 else contextlib.nullcontext()
            )
            with loop_cm:
                qraws = [
                    q_pool.tile([128, T * NJ], BF16, tag="qraw", name=f"qraw{nb}")
                    for nb in range(n_blocks)
                ]
                dps = []  # per block: (prev1, prev2) at the end of its chain

                def emit_tile(k, kl, nb):
                    ty = io_pool.tile([128, TLO * C], F32, tag="y", bufs=4)
                    src = y4[k].rearrange("e (th tl) c -> (e th) (tl c)", th=THI)
                    nc.sync.dma_start(ty[:], src)

                    # 8 E bufs: lets ACT/Pool run ahead while block1's Z ops
                    # queue behind DP(block0) on the DVE.
                    te = z_pool.tile([128, TLO * C], BF16, tag="E", bufs=int(os.environ.get("CTC_EBUFS", "8")))
                    # write E = exp(y + PBIAS) transposed to (c, tl)-major
                    te_t = te[:].rearrange("p (c tl) -> p tl c", tl=TLO)
                    ty3 = ty[:].rearrange("p (tl c) -> p tl c", tl=TLO)
                    nc.scalar.activation(te_t, ty3, AF.Exp, bias=biasln[:, 0:1])

                    # Z: halve-add 128->64->32, then reduce over 32
                    e3c = te[:].rearrange("p (c tl) -> p c tl", c=C)
                    th1 = z_pool.tile([128, 64 * TLO], BF16, tag="H")
                    h3 = th1[:].rearrange("p (c tl) -> p c tl", c=64)
                    nc.vector.tensor_tensor(
                        out=h3, in0=e3c[:, 0:64, :], in1=e3c[:, 64:128, :],
                        op=AOP.add,
                    )
                    th2 = z_pool.tile([128, 32 * TLO], BF16, tag="H2")
                    h23 = th2[:].rearrange("p (c tl) -> p c tl", c=32)
                    nc.vector.tensor_tensor(
                        out=h23, in0=h3[:, 0:32, :], in1=h3[:, 32:64, :],
                        op=AOP.add,
                    )
                    h2r = th2[:].rearrange("p (c tl) -> p tl c", c=32)
                    nc.vector.tensor_reduce(
                        out=zall[:, k * TLO : (k + 1) * TLO], in_=h2r,
                        axis=mybir.AxisListType.X, op=AOP.add,
                    )

                    tgat = g_pool.tile([128, NIDX_G * TLO], BF16, tag="gat")
                    ncol = NIDX_G // 16
                    nc.gpsimd.ap_gather(
                        out_ap=tgat[:],
                        in_ap=te[:].rearrange("p (c tl) -> p c tl", c=C),
                        idxs_ap=gidx_all[:, k * GIDX_STRIDE : k * GIDX_STRIDE + ncol],
                        channels=128,
                        num_elems=C,
                        d=TLO,
                        num_idxs=NIDX_G,
                    )
                    return tgat

                def emit_fold(kl, nb, tgat):
                    # fold: (e,th) parts x (j,tl) -> [8 ex, th*528 + j*16 + tl].
                    # Emitted a few tiles late so its wait on the gather is
                    # already satisfied when it reaches the SP queue head
                    # (otherwise it stalls the next y-load behind it).
                    dst = qraws[nb][:].rearrange("p (th w) -> p th w", th=THI)
                    nc.sync.dma_start(
                        dst[kl * NEX : (kl + 1) * NEX], tgat[:, 0 : NJ * TLO]
                    )

                def gen_dp_scans(nb):
                    """All-DVE part of the DP for block nb, one op per yield
                    so it can be interleaved with streaming Z ops; the final
                    Ln runs after all streaming ACT work."""
                    p3 = qraws[nb][:].rearrange(
                        "p (th j tl) -> p th j tl", th=THI, j=NJ
                    )

                    def pcol(s):
                        j = 0 if s % 2 == 0 else 1 + (s - 1) // 2
                        return p3[:, :, j, :]

                    scan3d(
                        nc, dpA[:, 1 : T + 1], onehot[:], pcol(0), 0.0,
                        AOP.add, AOP.mult,
                    )
                    yield
                    tu1 = u_pool.tile([128, T], BF16, tag="U", name=f"tu1_{nb}")
                    nc.vector.scalar_tensor_tensor(
                        out=tu1[:], in0=onehot[:], scalar=1.0, in1=dpA[:, 0:T],
                        op0=AOP.mult, op1=AOP.add,
                    )
                    yield
                    scan3d(
                        nc, dpB[:, 1 : T + 1], tu1[:], pcol(1), 0.0,
                        AOP.add, AOP.mult,
                    )
                    yield
                    prev2, prev1 = dpA, dpB
                    for s in range(2, S):
                        cur = cde[(s - 2) % 3]
                        if s % 2 == 0:
                            d0 = prev1[:, 0:T]
                        else:
                            i = (s - 1) // 2
                            tu = u_pool.tile([128, T], BF16, tag="U")
                            nc.vector.scalar_tensor_tensor(
                                out=tu[:], in0=prev2[:, 0:T],
                                scalar=masks[nb][:, i : i + 1],
                                in1=prev1[:, 0:T], op0=AOP.mult, op1=AOP.add,
                            )
                            yield
                            d0 = tu[:]
                        scan3d(
                            nc, cur[:, 1 : T + 1], d0, pcol(s), 0.0,
                            AOP.add, AOP.mult,
                        )
                        yield
                        prev2, prev1 = prev1, cur

                    # fin = alpha[S-1] + alpha[S-2] at t=T-1 (DVE, tiny)
                    nc.vector.tensor_tensor(
                        out=fins[:, nb : nb + 1], in0=prev1[:, T : T + 1],
                        in1=prev2[:, T : T + 1], op=AOP.add,
                    )
                    yield

                # ---- streaming (folds delayed 3 tiles); DP0 interleaved ----
                FOLD_LAG = 3
                N_DP_OPS = 2 * S + L  # 97 DVE ops per block's DP
                pending = []
                dp0 = None
                for k in range(ntiles):
                    nb, kl = divmod(k, TPB)
                    pending.append((kl, nb, emit_tile(k, kl, nb)))
                    if len(pending) > FOLD_LAG:
                        emit_fold(*pending.pop(0))
                        if k - FOLD_LAG == TPB - 1:
                            dp0 = gen_dp_scans(0)
                    if dp0 is not None:
                        # ~8 DP ops per remaining block1 tile keeps the DVE
                        # queue alternating between DP(b0) and Z(b1) work.
                        per_tile = -(-N_DP_OPS // (ntiles - 1 - (TPB + FOLD_LAG - 1)))
                        for _ in range(per_tile):
                            if next(dp0, StopIteration) is StopIteration:
                                dp0 = None
                                break
                while dp0 is not None and next(dp0, StopIteration) is not StopIteration:
                    pass
                for args in pending:
                    emit_fold(*args)

                # ---- logZ: Ln pass, per-tile sums, PE-matmul partition fold
                nc.scalar.activation(lnz[:], zall[:], AF.Ln)
                lnz3 = lnz[:].rearrange("p (k tl) -> p k tl", tl=TLO)
                nc.vector.tensor_reduce(
                    out=logzacc[:], in_=lnz3, axis=mybir.AxisListType.X, op=AOP.add
                )
                # zsum_ps[e, k] = sum_th logzacc[(e,th), k] = lzsum(ex = k*8+e)
                nc.tensor.matmul(
                    out=zsum_ps[:], lhsT=sel[:], rhs=logzacc[:],
                    start=True, stop=True,
                )
                zsum_sb = z_pool.tile([NEX, ntiles], F32, tag="zsum_sb")
                nc.scalar.copy(zsum_sb[:], zsum_ps[:])
                sc_w = scratch[:].rearrange("(e k) -> e k", e=NEX)
                nc.sync.dma_start(sc_w, zsum_sb[:])
                # rb[nb] enumerates (kl, e) = block-local example order
                rb = scratch[:].rearrange(
                    "(e nb kl) -> nb kl e", e=NEX, nb=n_blocks
                )
                for nb in range(n_blocks):
                    nc.sync.dma_start(lzs[:, nb : nb + 1], rb[nb])

                for _ in gen_dp_scans(1):
                    pass

                # ---- final loss: one Ln over both blocks' fins, subtract ----
                lfin = z_pool.tile([128, n_blocks], F32, tag="lfin")
                nc.scalar.activation(lfin[:], fins[:], AF.Ln)
                tloss = z_pool.tile([128, n_blocks], F32, tag="loss")
                nc.vector.tensor_tensor(
                    out=tloss[:], in0=lzs[:], in1=lfin[:], op=AOP.subtract
                )
                for nb in range(n_blocks):
                    nc.sync.dma_start(
                        loss_out[nb * 128 : (nb + 1) * 128], tloss[:, nb : nb + 1]
                    )


def _force_combined_act_table(nc):
    """Trim Exp/Ln from all act-function sets except the one that holds both,
    so the table-load pass picks the combined set and never reloads between
    the Exp (streaming) and Ln (logZ / final) activations."""
    from concourse.hw_specs import get_activation_tables

    tabs = get_activation_tables(nc.m.arch)
    combined = None
    for name, s in tabs.items():
        if AF.Exp in s and AF.Ln in s:
            combined = name
            break
    if combined is None:
        return
    for name, s in tabs.items():
        if name != combined:
            s.discard(AF.Exp)
            s.discard(AF.Ln)


import os

def _build_program(repeat=1):
    nc = bacc.Bacc("TRN2", num_devices=N_CORES, enable_partition_id=False)
    if not os.environ.get("CTC_NO_ACTFIX"):
        _force_combined_act_table(nc)
    y_in = nc.dram_tensor("y", [B_CORE, T, C], F32, kind="ExternalInput").ap()
    gidx_in = nc.dram_tensor(
        "gidx", [128, NTILES * GIDX_STRIDE], I16, kind="ExternalInput"
    ).ap()
    mask_in = nc.dram_tensor(
        "mask", [N_BLOCKS, 128, L], F32, kind="ExternalInput"
    ).ap()
    sel_in = nc.dram_tensor("sel", [128, NEX], F32, kind="ExternalInput").ap()
    loss_out = nc.dram_tensor("loss", [B_CORE], F32, kind="ExternalOutput").ap()
    build_ctc(nc, loss_out, y_in, gidx_in, mask_in, sel_in, repeat=repeat)
    nc.compile()
    return nc


def kernel(y_true: np.ndarray, y_pred: np.ndarray):
    y_true = np.asarray(y_true)
    y_pred = np.ascontiguousarray(np.asarray(y_pred, dtype=np.float32))
    assert y_pred.shape == (B_FULL, T, C) and y_true.shape == (B_FULL, L)

    nc = _build_program()
    in_maps = []
    for core in range(N_CORES):
        sl = slice(core * B_CORE, (core + 1) * B_CORE)
        yt = y_true[sl]
        in_maps.append(
            {
                "y": y_pred[sl],
                "gidx": make_gidx(yt),
                "mask": make_mask(yt),
                "sel": make_sel(),
            }
        )
    res = bass_utils.run_bass_kernel_spmd(
        nc, in_maps, core_ids=list(range(N_CORES))
    )
    loss = np.concatenate([r["loss"] for r in res.results])
    return loss.astype(np.float32)


if __name__ == "__main__":
    rng = np.random.default_rng(0)
    yp = rng.standard_normal((B_FULL, T, C)).astype(np.float32)
    yt = rng.integers(1, C, (B_FULL, L)).astype(np.int32)
    out = kernel(yt, yp)
    print(out.shape, out[:4])


# revision 23
# speedup vs baseline: 217.9244x; 1.0356x over previous
"""CTC loss Trainium2 kernel (Bass/Tile), pure data-parallel over 8 NeuronCores.

Contract: kernel(y_true [2048,32] i32, y_pred [2048,256,128] f32) -> loss [2048] f32.

Algorithm per core (256 examples = 2 blocks of 128, examples on partitions):
  Stream y in 32 tiles [128=(8 ex, 16 t_hi), 2048=(16 t_lo, 128 c)]:
    E = exp(y + PBIAS)             ACT -> bf16, written (c, tl)-major
    Z[b,t] = sum_c E               DVE: two halve-adds (bf16 2x) + reduce-32
    gather E at the 33 classes     GPSIMD ap_gather (per-16-partition idx)
    fold gather -> q[128 ex, th*528 + j*16 + tl]   SBUF->SBUF DMA (PE queue)
  DP(block0) emitted right after block0's 16 tiles so it overlaps block1
  streaming; only the tiny final-column adds run there, Ln/subtract deferred.
  logZ: one Ln pass over all Z, per-tile sums, then a PE matmul against a
  one-hot [128,8] selector folds the 16 t_hi partitions per example
  (replaces the strided DRAM bounce); tiny [8,32] bounce for the partition
  rearrange back to [128,1] per block.
  Per block of 128 examples:
    p'' = exp(q + PBIAS)  (PBIAS recenters alpha drift)
    CTC forward DP as 65 tensor_tensor_scan ops along time t=0..255:
      alpha_t[s] = (u_t + alpha_{t-1}[s]) * p''_t[s]
      u = alpha[s-1] (+ mask_i * alpha[s-2] for odd s>=3) via scalar_tensor_tensor;
      t=0 init injected through a one-hot drive column.
    loss = sum_t log Z_t + T*PBIAS - log(alpha_T[63] + alpha_T[64])
"""
import contextlib
import sys

sys.path.insert(0, "/opt/trn_rl_repo")
import numpy as np

import concourse.bacc as bacc
import concourse.mybir as mybir
import concourse.tile as tile
from concourse import bass_utils

F32 = mybir.dt.float32
BF16 = mybir.dt.bfloat16
I16 = mybir.dt.int16
AOP = mybir.AluOpType
AF = mybir.ActivationFunctionType

N_CORES = 8
B_FULL = 2048
C = 128
T = 256
L = 32
NJ = L + 1       # 33 gathered classes: [blank] + labels
S = 2 * L + 1    # 65 CTC states
NEX = 8          # examples per tile
THI = 16         # t-high bits on partitions
TLO = 16         # t-low bits in free dim
TPB = 16         # tiles per block
N_BLOCKS = 2     # blocks per core (256 examples)
UNROLL = 2       # kernel iterations per For_i trip (the trip barrier is
                 # all-engine; unrolling lets iteration i+1's streaming
                 # overlap iteration i's DP tail)
B_CORE = N_BLOCKS * 128
NTILES = N_BLOCKS * TPB
PBIAS = -0.95    # p'' = exp(y + PBIAS); balances FTZ-dip vs overflow margins
LOSS_CONST = -T * PBIAS


NIDX_G = 48  # 33 classes + 15 pad; ap_gather idx table is 16-partition-wrapped


GIDX_STRIDE = 8  # i16 cols per tile: 3 used + 5 pad so each tile's idx
                 # slice starts 16-byte aligned (HW gather requirement)


def make_gidx(y_true_shard: np.ndarray) -> np.ndarray:
    """[128, ntiles * GIDX_STRIDE] int16 ap_gather class indices, all tiles
    packed per partition (16B-aligned slices) so one DMA loads everything."""
    B = y_true_shard.shape[0]
    ntiles = B // NEX
    cls = np.concatenate(
        [np.zeros((B, 1), np.int64), y_true_shard.astype(np.int64)], axis=1
    )
    gidx = np.zeros((128, ntiles * GIDX_STRIDE), np.int16)
    i = np.arange(NIDX_G)
    prow, pcol = i % 16, i // 16
    for k in range(ntiles):
        for e in range(NEX):
            vals = np.zeros(NIDX_G, np.int64)
            vals[:NJ] = cls[k * NEX + e]
            gidx[16 * e + prow, k * GIDX_STRIDE + pcol] = vals
    return gidx


def make_sel() -> np.ndarray:
    """[128, NEX] f32 one-hot: sel[p, e] = (p // 16 == e); PE-matmul selector
    that folds the 16 t_hi partitions of each example."""
    p = np.arange(128)
    return (p[:, None] // 16 == np.arange(NEX)[None, :]).astype(np.float32)


def make_mask(y_true_shard: np.ndarray) -> np.ndarray:
    """[nblocks, 128, L] f32 skip masks: mask[b, i] = (lab_i != lab_{i-1})."""
    B = y_true_shard.shape[0]
    m = np.zeros((B, L), np.float32)
    m[:, 1:] = (y_true_shard[:, 1:] != y_true_shard[:, :-1]).astype(np.float32)
    return m.reshape(B // 128, 128, L)


def scan3d(nc, out, data0, data1, initial, op0, op1):
    """tensor_tensor_scan allowing a 3-D strided data1 AP (verified on HW:
    the recurrence chains across AP dims in enumeration order)."""
    eng = nc.vector
    return eng.add_instruction(
        mybir.InstTensorScalarPtr(
            name=nc.get_next_instruction_name(),
            is_tensor_tensor_scan=True,
            is_scalar_tensor_tensor=True,
            op0=op0,
            op1=op1,
            ins=[
                eng.lower_ap(data0),
                eng.lower_ap_or_imm(initial),
                eng.lower_ap(data1),
            ],
            outs=[eng.lower_ap(out)],
        )
    )


def build_ctc(nc, loss_out, y_in, gidx_in, mask_in, sel_in, n_blocks=N_BLOCKS, repeat=1):
    ntiles = n_blocks * TPB
    with tile.TileContext(nc) as tc:
        with (
            tc.tile_pool(name="io", bufs=3) as io_pool,
            tc.tile_pool(name="ztmp", bufs=2) as z_pool,
            tc.tile_pool(name="gat", bufs=2) as g_pool,
            tc.tile_pool(name="qraw", bufs=2) as q_pool,
            tc.tile_pool(name="persist", bufs=1) as pp,
            tc.tile_pool(name="ubuf", bufs=2) as u_pool,
            tc.tile_pool(name="dram", bufs=1, space="DRAM") as d_pool,
            tc.psum_pool(name="psum", bufs=1) as ps_pool,
        ):
            gidx_all = pp.tile([128, ntiles * GIDX_STRIDE], I16, tag="gidx")
            dpA = pp.tile([128, T + 1], BF16, tag="dpA")
            dpB = pp.tile([128, T + 1], BF16, tag="dpB")
            cde = [
                pp.tile([128, T + 1], BF16, tag=f"cde{i}", name=f"cde{i}")
                for i in range(3)
            ]
            onehot = pp.tile([128, T], BF16, tag="onehot")
            sel = pp.tile([128, NEX], F32, tag="sel")
            masks = [
                pp.tile([128, L], F32, tag=f"mask{nb}", name=f"mask{nb}")
                for nb in range(n_blocks)
            ]
            biasln = pp.tile([128, 1], F32, tag="biasln")

            nc.vector.memset(biasln[:], PBIAS)
            nc.vector.memset(onehot[:], 0.0)
            nc.vector.memset(onehot[:, 0:1], 1.0)
            for b in (dpA, dpB, *cde):
                nc.vector.memset(b[:, 0:1], 0.0)
            for nb in range(n_blocks):
                nc.sync.dma_start(masks[nb][:], mask_in[nb])
            nc.sync.dma_start(gidx_all[:], gidx_in)
            nc.sync.dma_start(sel[:], sel_in)

            # Touch Exp and Ln once before the loop so activation-table
            # loads don't land inside the repeat body.
            warm = pp.tile([128, 1], F32, tag="warm")
            nc.scalar.activation(warm[:], biasln[:], AF.Exp)
            nc.scalar.activation(warm[:], warm[:], AF.Ln)
            # Touch ap_gather too: hoists the gpsimd library load (which
            # otherwise lands inside the loop body, behind the per-iteration
            # all-engine barrier).
            warmg_in = pp.tile([128, 2 * C], BF16, tag="warmg_in")
            warmg_out = pp.tile([128, 2 * NIDX_G], BF16, tag="warmg_out")
            nc.vector.memset(warmg_in[:], 0.0)
            nc.gpsimd.ap_gather(
                out_ap=warmg_out[:],
                in_ap=warmg_in[:].rearrange("p (c d) -> p c d", c=C),
                idxs_ap=gidx_all[:, 0 : NIDX_G // 16],
                channels=128,
                num_elems=C,
                d=2,
                num_idxs=NIDX_G,
            )

            y4 = y_in.rearrange("(k e) t c -> k e t c", e=NEX)

            def emit_iteration():
                logzacc = pp.tile([128, ntiles], F32, tag="logzacc", bufs=2)
                zall = pp.tile([128, ntiles * TLO], F32, tag="zall", bufs=2)
                lnz = pp.tile([128, ntiles * TLO], F32, tag="lnz", bufs=2)
                scratch = d_pool.tile([NEX * ntiles], F32, tag="scratch", bufs=2)
                fins = pp.tile([128, n_blocks], F32, tag="fins", bufs=2)
                lzs = pp.tile([128, n_blocks], F32, tag="lzs", bufs=2)
                zsum_ps = ps_pool.tile([NEX, ntiles], F32, tag="zsum", bufs=2)
                qraws = [
                    q_pool.tile([128, T * NJ], BF16, tag="qraw",
                                name=f"qraw{nb}", bufs=2 * UNROLL)
                    for nb in range(n_blocks)
                ]

                def emit_tile(k, kl, nb):
                    ty = io_pool.tile([128, TLO * C], F32, tag="y", bufs=4)
                    src = y4[k].rearrange("e (th tl) c -> (e th) (tl c)", th=THI)
                    nc.sync.dma_start(ty[:], src)

                    # 8 E bufs: lets ACT/Pool run ahead while block1's Z ops
                    # queue behind DP(block0) on the DVE.
                    te = z_pool.tile([128, TLO * C], BF16, tag="E", bufs=int(os.environ.get("CTC_EBUFS", "8")))
                    # write E = exp(y + PBIAS) transposed to (c, tl)-major
                    te_t = te[:].rearrange("p (c tl) -> p tl c", tl=TLO)
                    ty3 = ty[:].rearrange("p (tl c) -> p tl c", tl=TLO)
                    nc.scalar.activation(te_t, ty3, AF.Exp, bias=biasln[:, 0:1])

                    # Z: halve-add 128->64->32, then reduce over 32
                    e3c = te[:].rearrange("p (c tl) -> p c tl", c=C)
                    th1 = z_pool.tile([128, 64 * TLO], BF16, tag="H")
                    h3 = th1[:].rearrange("p (c tl) -> p c tl", c=64)
                    nc.vector.tensor_tensor(
                        out=h3, in0=e3c[:, 0:64, :], in1=e3c[:, 64:128, :],
                        op=AOP.add,
                    )
                    th2 = z_pool.tile([128, 32 * TLO], BF16, tag="H2")
                    h23 = th2[:].rearrange("p (c tl) -> p c tl", c=32)
                    nc.vector.tensor_tensor(
                        out=h23, in0=h3[:, 0:32, :], in1=h3[:, 32:64, :],
                        op=AOP.add,
                    )
                    h2r = th2[:].rearrange("p (c tl) -> p tl c", c=32)
                    nc.vector.tensor_reduce(
                        out=zall[:, k * TLO : (k + 1) * TLO], in_=h2r,
                        axis=mybir.AxisListType.X, op=AOP.add,
                    )

                    tgat = g_pool.tile([128, NIDX_G * TLO], BF16, tag="gat")
                    ncol = NIDX_G // 16
                    nc.gpsimd.ap_gather(
                        out_ap=tgat[:],
                        in_ap=te[:].rearrange("p (c tl) -> p c tl", c=C),
                        idxs_ap=gidx_all[:, k * GIDX_STRIDE : k * GIDX_STRIDE + ncol],
                        channels=128,
                        num_elems=C,
                        d=TLO,
                        num_idxs=NIDX_G,
                    )
                    return tgat

                def emit_fold(kl, nb, tgat):
                    # fold: (e,th) parts x (j,tl) -> [8 ex, th*528 + j*16 + tl].
                    # Emitted a few tiles late so its wait on the gather is
                    # already satisfied when it reaches the SP queue head
                    # (otherwise it stalls the next y-load behind it).
                    dst = qraws[nb][:].rearrange("p (th w) -> p th w", th=THI)
                    nc.sync.dma_start(
                        dst[kl * NEX : (kl + 1) * NEX], tgat[:, 0 : NJ * TLO]
                    )

                def gen_dp_scans(nb):
                    """All-DVE part of the DP for block nb, one op per yield
                    so it can be interleaved with streaming Z ops; the final
                    Ln runs after all streaming ACT work."""
                    p3 = qraws[nb][:].rearrange(
                        "p (th j tl) -> p th j tl", th=THI, j=NJ
                    )

                    def pcol(s):
                        j = 0 if s % 2 == 0 else 1 + (s - 1) // 2
                        return p3[:, :, j, :]

                    scan3d(
                        nc, dpA[:, 1 : T + 1], onehot[:], pcol(0), 0.0,
                        AOP.add, AOP.mult,
                    )
                    yield
                    tu1 = u_pool.tile([128, T], BF16, tag="U", name=f"tu1_{nb}")
                    nc.vector.scalar_tensor_tensor(
                        out=tu1[:], in0=onehot[:], scalar=1.0, in1=dpA[:, 0:T],
                        op0=AOP.mult, op1=AOP.add,
                    )
                    yield
                    scan3d(
                        nc, dpB[:, 1 : T + 1], tu1[:], pcol(1), 0.0,
                        AOP.add, AOP.mult,
                    )
                    yield
                    prev2, prev1 = dpA, dpB
                    for s in range(2, S):
                        cur = cde[(s - 2) % 3]
                        if s % 2 == 0:
                            d0 = prev1[:, 0:T]
                        else:
                            i = (s - 1) // 2
                            tu = u_pool.tile([128, T], BF16, tag="U")
                            nc.vector.scalar_tensor_tensor(
                                out=tu[:], in0=prev2[:, 0:T],
                                scalar=masks[nb][:, i : i + 1],
                                in1=prev1[:, 0:T], op0=AOP.mult, op1=AOP.add,
                            )
                            yield
                            d0 = tu[:]
                        scan3d(
                            nc, cur[:, 1 : T + 1], d0, pcol(s), 0.0,
                            AOP.add, AOP.mult,
                        )
                        yield
                        prev2, prev1 = prev1, cur

                    # fin = alpha[S-1] + alpha[S-2] at t=T-1 (DVE, tiny)
                    nc.vector.tensor_tensor(
                        out=fins[:, nb : nb + 1], in0=prev1[:, T : T + 1],
                        in1=prev2[:, T : T + 1], op=AOP.add,
                    )
                    yield

                # ---- streaming (folds delayed 3 tiles); DP0 interleaved ----
                FOLD_LAG = 3
                N_DP_OPS = 2 * S + L  # 97 DVE ops per block's DP
                pending = []
                dp0 = None
                for k in range(ntiles):
                    nb, kl = divmod(k, TPB)
                    pending.append((kl, nb, emit_tile(k, kl, nb)))
                    if len(pending) > FOLD_LAG:
                        emit_fold(*pending.pop(0))
                        if k - FOLD_LAG == TPB - 1:
                            dp0 = gen_dp_scans(0)
                    if dp0 is not None:
                        # ~8 DP ops per remaining block1 tile keeps the DVE
                        # queue alternating between DP(b0) and Z(b1) work.
                        per_tile = -(-N_DP_OPS // (ntiles - 1 - (TPB + FOLD_LAG - 1)))
                        for _ in range(per_tile):
                            if next(dp0, StopIteration) is StopIteration:
                                dp0 = None
                                break
                while dp0 is not None and next(dp0, StopIteration) is not StopIteration:
                    pass
                for args in pending:
                    emit_fold(*args)

                # ---- logZ: Ln pass, per-tile sums, PE-matmul partition fold
                nc.scalar.activation(lnz[:], zall[:], AF.Ln)
                lnz3 = lnz[:].rearrange("p (k tl) -> p k tl", tl=TLO)
                nc.vector.tensor_reduce(
                    out=logzacc[:], in_=lnz3, axis=mybir.AxisListType.X, op=AOP.add
                )
                # zsum_ps[e, k] = sum_th logzacc[(e,th), k] = lzsum(ex = k*8+e)
                nc.tensor.matmul(
                    out=zsum_ps[:], lhsT=sel[:], rhs=logzacc[:],
                    start=True, stop=True,
                )
                zsum_sb = z_pool.tile([NEX, ntiles], F32, tag="zsum_sb")
                nc.scalar.copy(zsum_sb[:], zsum_ps[:])
                sc_w = scratch[:].rearrange("(e k) -> e k", e=NEX)
                nc.sync.dma_start(sc_w, zsum_sb[:])
                # rb[nb] enumerates (kl, e) = block-local example order
                rb = scratch[:].rearrange(
                    "(e nb kl) -> nb kl e", e=NEX, nb=n_blocks
                )
                for nb in range(n_blocks):
                    nc.sync.dma_start(lzs[:, nb : nb + 1], rb[nb])

                for _ in gen_dp_scans(1):
                    pass

                # ---- final loss: one Ln over both blocks' fins, subtract ----
                lfin = z_pool.tile([128, n_blocks], F32, tag="lfin")
                nc.scalar.activation(lfin[:], fins[:], AF.Ln)
                tloss = z_pool.tile([128, n_blocks], F32, tag="loss")
                nc.vector.tensor_tensor(
                    out=tloss[:], in0=lzs[:], in1=lfin[:], op=AOP.subtract
                )
                for nb in range(n_blocks):
                    nc.sync.dma_start(
                        loss_out[nb * 128 : (nb + 1) * 128], tloss[:, nb : nb + 1]
                    )

            loop_cm = (
                tc.For_i(0, repeat, 1) if repeat > 1 else contextlib.nullcontext()
            )
            with loop_cm:
                for _sub in range(UNROLL):
                    emit_iteration()


def _force_combined_act_table(nc):
    """Trim Exp/Ln from all act-function sets except the one that holds both,
    so the table-load pass picks the combined set and never reloads between
    the Exp (streaming) and Ln (logZ / final) activations."""
    from concourse.hw_specs import get_activation_tables

    tabs = get_activation_tables(nc.m.arch)
    combined = None
    for name, s in tabs.items():
        if AF.Exp in s and AF.Ln in s:
            combined = name
            break
    if combined is None:
        return
    for name, s in tabs.items():
        if name != combined:
            s.discard(AF.Exp)
            s.discard(AF.Ln)


import os

def _build_program(repeat=1):
    nc = bacc.Bacc("TRN2", num_devices=N_CORES, enable_partition_id=False)
    if not os.environ.get("CTC_NO_ACTFIX"):
        _force_combined_act_table(nc)
    y_in = nc.dram_tensor("y", [B_CORE, T, C], F32, kind="ExternalInput").ap()
    gidx_in = nc.dram_tensor(
        "gidx", [128, NTILES * GIDX_STRIDE], I16, kind="ExternalInput"
    ).ap()
    mask_in = nc.dram_tensor(
        "mask", [N_BLOCKS, 128, L], F32, kind="ExternalInput"
    ).ap()
    sel_in = nc.dram_tensor("sel", [128, NEX], F32, kind="ExternalInput").ap()
    loss_out = nc.dram_tensor("loss", [B_CORE], F32, kind="ExternalOutput").ap()
    build_ctc(nc, loss_out, y_in, gidx_in, mask_in, sel_in, repeat=repeat)
    nc.compile()
    return nc


def kernel(y_true: np.ndarray, y_pred: np.ndarray):
    y_true = np.asarray(y_true)
    y_pred = np.ascontiguousarray(np.asarray(y_pred, dtype=np.float32))
    assert y_pred.shape == (B_FULL, T, C) and y_true.shape == (B_FULL, L)

    nc = _build_program()
    in_maps = []
    for core in range(N_CORES):
        sl = slice(core * B_CORE, (core + 1) * B_CORE)
        yt = y_true[sl]
        in_maps.append(
            {
                "y": y_pred[sl],
                "gidx": make_gidx(yt),
                "mask": make_mask(yt),
                "sel": make_sel(),
            }
        )
    res = bass_utils.run_bass_kernel_spmd(
        nc, in_maps, core_ids=list(range(N_CORES))
    )
    loss = np.concatenate([r["loss"] for r in res.results])
    return loss.astype(np.float32)


if __name__ == "__main__":
    rng = np.random.default_rng(0)
    yp = rng.standard_normal((B_FULL, T, C)).astype(np.float32)
    yt = rng.integers(1, C, (B_FULL, L)).astype(np.int32)
    out = kernel(yt, yp)
    print(out.shape, out[:4])
